# revision 11
# baseline (speedup 1.0000x reference)
"""Trainium2 Bass kernel for nn_DeChunkLayer (ragged_sequence).

Math (per batch row):
    p[c]     = clip(boundary_prob[take_idx[c]], EPS, 1-EPS)
    decay    = 1 - p, decay[0] = 0
    weighted = p * hidden, weighted[0] = hidden[0]
    smoothed[c] = decay[c] * smoothed[c-1] + weighted[c]      (EMA scan over C)
    chunk_id = clip(cumsum(boundary_mask) - 1, 0, C-1)
    out[l]   = smoothed[chunk_id[l]]

v5 vs v3 (baseline):
  - Output is bf16 in a partition-major DRAM layout [128, 64, D] (token
    l at [l%128, l//128]): every out-write descriptor covers 8KB
    contiguous DRAM (vs 2KB strided), and the bf16->f32 upconvert stage
    is gone entirely -- the host upcasts/unpermutes. Phase 3 is pure DMA.
  - smoothed DRAM scratch is p-major so the writeback is one contiguous
    16KB run per partition; gather rows are ck' = (c%128)*16 + c//128.
  - The p gather runs as 2 DMAGatherAnt calls over a host-padded
    [L, 64] prob table (elem=256B): no INDIRECT1D ucode, so GpSimd pays
    the gather-ant library load once (a tiny warmup call hides it) and
    the ~9us library-reload stall before the output-gather preps is gone.
  - The EMA scan runs in c-half-major order with a carry fold, so half A
    of smoothed lands early and gather call 0 (tokens < 1024 only
    reference chunks < 1024, since chunk_id[l] <= l) fires early.
  - p broadcast: 16 K=1 matmuls straight from the transposed p row (one
    partition each) -- no DRAM bounce.
  - GpSimd stream: [warmup, p-gather x2, preps 0-3, T(1)@wbA, prep4,
    T(3)@wbB, prep5, T(1), prep6, T(1), prep7, T(2)] so desc-gen never
    idles and each gather fires as soon as its prep + data are ready.
Tile does not emit RAW waits through DRAM scratch tensors, so the
triggers carry explicit sync deps on the smoothed writeback DMAs.
"""

import numpy as np

import concourse.bass as bass
import concourse.bacc as bacc
import concourse.mybir as mybir
import concourse.tile as tile
from concourse.bass_utils import run_bass_kernel_spmd

B, L, C, D = 8, 8192, 2048, 512
EPS = 1e-4
P = 128
NDG = D // P          # 4 partition groups of the transposed layout
NCB = C // P          # 16 c-blocks of 128
QW = 16               # wrap width of the dma_gather index layout
FW = L // QW          # 512 free positions in the wrapped layout
NGC = 8               # output dma_gather calls (num_idxs=1024 is a HW limit)
IPC = L // NGC        # 1024 indices per call
CH = C // 2           # half width of the scan split
PE = 64               # padded prob row: 64 f32 = 256B (min gather elem)

F32 = mybir.dt.float32
BF16 = mybir.dt.bfloat16
I16 = mybir.dt.int16
I32 = mybir.dt.int32
U8 = mybir.dt.uint8
COPY = mybir.ActivationFunctionType.Copy

_CACHED_NC = None


def build_nc() -> bacc.Bacc:
    nc = bacc.Bacc("TRN2", target_bir_lowering=False, debug=False)

    hidden_t = nc.dram_tensor("hidden_t", [D, C], F32, kind="ExternalInput")
    maskw_d = nc.dram_tensor("maskw", [QW, FW], U8, kind="ExternalInput")
    probp_d = nc.dram_tensor("probp", [L, PE], F32, kind="ExternalInput")
    tidx16_d = nc.dram_tensor("tidx16", [P, C // QW], I16, kind="ExternalInput")
    ident_d = nc.dram_tensor("ident", [P, P], F32, kind="ExternalInput")
    identbf_d = nc.dram_tensor("identbf", [P, P], BF16, kind="ExternalInput")
    uti_d = nc.dram_tensor("uti", [QW, QW], F32, kind="ExternalInput")
    out_d = nc.dram_tensor("out", [P, L // P, D], BF16, kind="ExternalOutput")

    with tile.TileContext(nc) as tc:
        with (
            tc.tile_pool(name="persist", bufs=1) as pp,
            tc.tile_pool(name="gather", bufs=NGC) as gp,
            tc.tile_pool(name="psum", bufs=2, space="PSUM") as psp,
        ):
            # psone (ck16 + p-row PSUM) closes before the 4-bank pb tile
            # is allocated (8 PSUM banks total).
            psone_cm = tc.tile_pool(name="psone", bufs=1, space="PSUM")
            psone = psone_cm.__enter__()
            # manual semaphores: one per output gather (the 16 per-queue
            # completion incs of call k don't distinguish calls on a shared
            # counter) plus one per p-gather call. alloc does not clear, so
            # clear them up front on the (idle) sync engine.
            dma_sems = [nc.alloc_semaphore(f"gdma{k}") for k in range(NGC)]
            psems = [nc.alloc_semaphore(f"pdma{j}") for j in range(2)]
            for s in dma_sems + psems:
                nc.sync.sem_clear(s)

            # ---------------- input loads ----------------
            tidx16_sb = pp.tile([P, C // QW], I16)
            nc.sync.dma_start(tidx16_sb[:], tidx16_d[:])
            maskw_sb = pp.tile([QW, FW], U8)
            nc.scalar.dma_start(maskw_sb[:], maskw_d[:])

            # constants from the host (keeps GpSimd free for desc-gen)
            ident = pp.tile([P, P], F32)
            nc.sync.dma_start(ident[:], ident_d[:])
            ident_bf = pp.tile([P, P], BF16)
            nc.sync.dma_start(ident_bf[:], identbf_d[:])
            uti = pp.tile([QW, QW], F32)
            nc.sync.dma_start(uti[:], uti_d[:])

            # hidden loads issued up front (queues are idle in phase 1)
            hts = []
            for dg in range(NDG):
                ht = pp.tile([P, C], F32, tag=f"ht{dg}")
                nc.sync.dma_start(ht[:], hidden_t[dg * P:(dg + 1) * P, :])
                hts.append(ht)

            # ---------------- gpsimd: p gather (DMAGatherAnt) ----------
            # warmup: pays the gather-ant Q7 library load (~9us) before the
            # real calls; gathers 128 garbage rows into a scratch tile.
            warm = pp.tile([P, 1, PE], F32)
            nc.gpsimd.dma_gather(
                out_ap=warm[:], in_ap=probp_d[:],
                idxs_ap=tidx16_sb[:, 0:8],
                num_idxs=P, num_idxs_reg=P, elem_size=PE,
            ).then_inc(psems[0], 16)
            # p2big[p, j, 0] = prob[take_idx[j*128 + p]] (elem pad of 64)
            p2big = pp.tile([P, NCB, PE], F32)
            pg0 = nc.gpsimd.dma_gather(
                out_ap=p2big[:, 0:NCB // 2, :], in_ap=probp_d[:],
                idxs_ap=tidx16_sb[:, 0:C // 2 // QW],
                num_idxs=C // 2, num_idxs_reg=C // 2, elem_size=PE,
            )
            pg0.then_inc(psems[0], 16)
            pg1 = nc.gpsimd.dma_gather(
                out_ap=p2big[:, NCB // 2:NCB, :], in_ap=probp_d[:],
                idxs_ap=tidx16_sb[:, C // 2 // QW:C // QW],
                num_idxs=C // 2, num_idxs_reg=C // 2, elem_size=PE,
            )
            pg1.then_inc(psems[1], 16)

            ones16 = pp.tile([1, QW], F32)
            nc.vector.memset(ones16[:], 1.0)
            ones161 = pp.tile([QW, 1], F32)
            nc.vector.memset(ones161[:], 1.0)
            ones1 = pp.tile([1, FW], F32)
            nc.vector.memset(ones1[:], 1.0)

            # ---------------- chunk ids (early: preps need ckrep) ----------
            # CK16[q, j] = chunk_id[16j + q]. Host passes maskw[q, j] =
            # mask[16j + q]. Within-column inclusive prefix over q via the
            # 16x16 inclusive triangular matmul; across-column exclusive
            # prefix of the column sums via a 1-partition scan, broadcast
            # into the same PSUM bank with a ones matmul.
            maskwf = pp.tile([QW, FW], F32)
            nc.vector.tensor_copy(maskwf[:], maskw_sb[:])
            ps16 = psone.tile([QW, FW], F32, space="PSUM", tag="ps16")
            nc.tensor.matmul(ps16[:], lhsT=uti[:], rhs=maskwf[:],
                             start=True, stop=False)
            cs_ps = psone.tile([1, FW], F32, space="PSUM", tag="cs")
            nc.tensor.matmul(cs_ps[:], lhsT=ones161[:], rhs=maskwf[:],
                             start=True, stop=True)
            colsb = pp.tile([1, FW], F32)
            nc.vector.tensor_copy(colsb[:], cs_ps[:])
            exc0 = pp.tile([1, FW], F32)
            nc.vector.tensor_tensor_scan(
                exc0[:], ones1[:], colsb[:],
                0.0, mybir.AluOpType.mult, mybir.AluOpType.add)
            nc.vector.tensor_tensor(exc0[:], exc0[:], colsb[:],
                                    mybir.AluOpType.subtract)
            nc.tensor.matmul(ps16[:], lhsT=ones16[:], rhs=exc0[:],
                             start=False, stop=True)
            ck = pp.tile([QW, FW], F32)
            nc.vector.tensor_scalar(ck[:], ps16[:], -1.0, None,
                                    mybir.AluOpType.add)
            nc.vector.tensor_scalar(ck[:], ck[:], 0.0, float(C - 1),
                                    mybir.AluOpType.max, mybir.AluOpType.min)
            # remap to the p-major smoothed layout: chunk c lives at row
            # (c%128)*16 + c//128 of the [2048, D] scratch view.
            cki = pp.tile([QW, FW], I32)
            nc.vector.tensor_copy(cki[:], ck[:])
            chi = pp.tile([QW, FW], I32)
            nc.vector.tensor_scalar(chi[:], cki[:], 7, None,
                                    mybir.AluOpType.logical_shift_right)
            nc.vector.tensor_scalar(cki[:], cki[:], 127, None,
                                    mybir.AluOpType.bitwise_and)
            nc.vector.tensor_scalar(cki[:], cki[:], 4, None,
                                    mybir.AluOpType.logical_shift_left)
            nc.vector.tensor_tensor(cki[:], cki[:], chi[:],
                                    mybir.AluOpType.add)
            ck16 = pp.tile([QW, FW], I16)
            nc.vector.tensor_copy(ck16[:], cki[:])
            # replicate to all 8 GPSIMD core groups (cross-partition copies)
            ckrep = pp.tile([P, FW], I16)
            for cgrp in range(P // QW):
                nc.scalar.dma_start(ckrep[cgrp * QW:(cgrp + 1) * QW, :], ck16[:])

            # prepared output gathers: desc-gen on GpSimd right after the p
            # gather (only needs ckrep); the DMAs fire at the triggers below.
            # sm_scratch is a raw (untracked) DRAM tensor so the preps do not
            # inherit the smoothed-writeback RAW dep -- that dep is carried
            # manually by the triggers. Layout is p-major: [p, ci, d].
            sm_dram = nc.dram_tensor("sm_scratch", [P, NCB, D], BF16,
                                     kind="Internal")
            sm_flat = sm_dram[:].rearrange("p ci d -> (p ci) d")
            gs = []

            def emit_prep(k):
                g = gp.tile([P, IPC // P, D], BF16, tag="g")
                nc.gpsimd.dma_gather(
                    out_ap=g[:], in_ap=sm_flat,
                    idxs_ap=ckrep[:, k * (FW // NGC):(k + 1) * (FW // NGC)],
                    num_idxs=IPC, num_idxs_reg=IPC, elem_size=D,
                    prepare_only=True, sem=dma_sems[k])
                gs.append(g)

            # GpSimd stream order == emission order: the first NEARLY preps
            # run while the scan pipeline computes; the remaining preps are
            # emitted between triggers so desc-gen overlaps phase-3 DMA
            # instead of stalling behind writeback sem waits.
            NEARLY = 4
            for k in range(NEARLY):
                emit_prep(k)

            # ---------------- p row -> PSUM broadcast ----------------
            p2 = p2big[:, :, 0:1].rearrange("p j o -> p (j o)")  # [128, 16]
            nc.vector.wait_ge(psems[0], 32)   # warmup + p-gather call 0
            nc.vector.wait_ge(psems[1], 16)   # p-gather call 1
            nc.vector.tensor_scalar(p2, p2, EPS, 1.0 - EPS,
                                    mybir.AluOpType.max,
                                    mybir.AluOpType.min)
            nc.vector.memset(p2big[0:1, 0:1, 0:1], 1.0)  # w[0]=h[0], d[0]=0
            pT_ps = psone.tile([NCB, P], F32, space="PSUM", tag="prow")
            nc.tensor.transpose(pT_ps[:], p2, ident[:])
            pT = pp.tile([NCB, P], F32)
            nc.scalar.copy(pT[:], pT_ps[:])
            prow = pp.tile([1, C], F32)
            p_dram = nc.dram_tensor("p_row_scratch", [1, C], F32, kind="Internal")
            w_pr = nc.scalar.dma_start(
                p_dram[:].rearrange("o (j q) -> (o j) q", j=NCB), pT[:])
            r_pr = nc.scalar.dma_start(prow[:], p_dram[:])
            bass._add_dep_helper(r_pr.ins, w_pr.ins, sync=True,
                                 reason="p row bounce raw")
            ones128 = pp.tile([1, P], F32)
            nc.vector.memset(ones128[:], 1.0)
            psone_cm.__exit__(None, None, None)
            pbp_cm = tc.tile_pool(name="pbh", bufs=1, space="PSUM")
            pbp = pbp_cm.__enter__()
            pb_ps = pbp.tile([P, C], F32, space="PSUM", tag="pb")
            for n in range(C // 512):
                nc.tensor.matmul(pb_ps[:, n * 512:(n + 1) * 512],
                                 lhsT=ones128[:],
                                 rhs=prow[:, n * 512:(n + 1) * 512],
                                 start=True, stop=True)
            db = pp.tile([P, C], BF16)
            nc.scalar.activation(db[:], pb_ps[:], COPY, bias=1.0, scale=-1.0)

            # ---------------- EMA scan, c-half-major with carry fold -------
            # half A (chunks < 1024) completes for all 4 d-groups first so
            # its writeback (and gather call 0) can fire early.
            sm_sb = pp.tile([P, NCB * D], BF16)  # [c-in-block, (c-block, d)]
            wts, sts = [], []
            for dg in range(NDG):
                wt = pp.tile([P, C], BF16, tag=f"wt{dg}")
                st = pp.tile([P, C], BF16, tag=f"st{dg}")
                wts.append(wt)
                sts.append(st)

            def transpose_out(dg, ci, engine):
                ps = psp.tile([P, P], BF16, space="PSUM", tag="tps")
                nc.tensor.transpose(ps[:], sts[dg][:, ci * P:(ci + 1) * P],
                                    ident_bf[:])
                dst = sm_sb[:, ci * D + dg * P: ci * D + (dg + 1) * P]
                engine(dst, ps[:])

            for half in range(2):
                lo, hi = half * CH, (half + 1) * CH
                for dg in range(NDG):
                    wt, st = wts[dg], sts[dg]
                    if half == 0:
                        nc.vector.tensor_tensor(wt[:], hts[dg][:], pb_ps[:],
                                                mybir.AluOpType.mult)
                    else:
                        # fold the half-A carry into the first weighted col
                        carry = pp.tile([P, 1], F32, tag=f"cr{dg}")
                        nc.vector.tensor_tensor(
                            carry[:], st[:, CH - 1:CH], db[:, CH:CH + 1],
                            mybir.AluOpType.mult)
                        nc.vector.tensor_tensor(
                            wt[:, CH:CH + 1], wt[:, CH:CH + 1], carry[:],
                            mybir.AluOpType.add)
                    nc.vector.tensor_tensor_scan(
                        st[:, lo:hi], db[:, lo:hi], wt[:, lo:hi], 0.0,
                        mybir.AluOpType.mult, mybir.AluOpType.add)
                    # transpose this (half, dg)'s 8 c-blocks while the next
                    # scan runs; copies go to Scalar (DVE keeps scanning),
                    # split with DVE for the last d-group of each half so
                    # the half's writeback isn't gated on a scalar backlog.
                    for ci in range(half * (NCB // 2), (half + 1) * (NCB // 2)):
                        if dg == NDG - 1 and ci % 2 == 0:
                            transpose_out(dg, ci, nc.vector.tensor_copy)
                        else:
                            transpose_out(dg, ci, nc.scalar.copy)
                # writeback this half: contiguous 8KB per partition
                hb, he = half * (NCB // 2), (half + 1) * (NCB // 2)
                w = nc.sync.dma_start(
                    sm_dram[:, hb:he, :],
                    sm_sb[:].rearrange("p (ci d) -> p ci d", d=D)[:, hb:he, :])
                if half == 0:
                    w_sm_a = w
                else:
                    w_sm_b = w

            # ---------------- output expansion ----------------
            # triggers fire each prepared gather once the smoothed data it
            # can reference has landed: call 0 only touches chunks < 1024
            # (chunk_id[l] <= l), later calls may touch anything.
            trig_of = []

            def emit_trigger(count, need_b):
                tr = nc.gpsimd.trigger_dma(count=count)
                bass._add_dep_helper(tr.ins, w_sm_a.ins, sync=True,
                                     reason="smoothed gather raw a")
                if need_b:
                    bass._add_dep_helper(tr.ins, w_sm_b.ins, sync=True,
                                         reason="smoothed gather raw b")
                trig_of.extend([tr] * count)
                return tr

            emit_trigger(1, need_b=False)   # call 0
            emit_prep(4)
            emit_trigger(3, need_b=True)    # calls 1-3
            emit_prep(5)
            emit_trigger(1, need_b=True)    # call 4
            emit_prep(6)
            emit_trigger(1, need_b=True)    # call 5
            emit_prep(7)
            emit_trigger(2, need_b=True)    # calls 6-7

            for k in range(NGC):
                # Tile's auto DMASW wait is satisfied by the prep-time
                # pre-bump, not the gather's completion; the baked per-call
                # sem is the real data-ready signal. The no-sync edge on the
                # trigger keeps the scheduler from hoisting this wait ahead
                # of the phase-1 work on its engine.
                od = nc.sync.dma_start(
                    out_d[:, k * (IPC // P):(k + 1) * (IPC // P), :],
                    gs[k][:])
                od._wait_ge(dma_sems[k], 16)
                bass._add_dep_helper(od.ins, trig_of[k].ins, sync=False,
                                     reason="out write after trigger")
            pbp_cm.__exit__(None, None, None)

    nc.compile()
    return nc


def _shard_inputs(hidden_states, boundary_mask, boundary_prob, take_idx):
    import ml_dtypes
    hidden_states = np.asarray(hidden_states, dtype=np.float32)
    boundary_mask = np.asarray(boundary_mask)
    boundary_prob = np.asarray(boundary_prob, dtype=np.float32)
    take_idx = np.asarray(take_idx)
    ident = np.eye(P, dtype=np.float32)
    identbf = np.eye(P).astype(ml_dtypes.bfloat16)
    uti = np.triu(np.ones((QW, QW), dtype=np.float32))
    in_maps = []
    for b in range(B):
        probp = np.zeros((L, PE), dtype=np.float32)
        probp[:, 0] = boundary_prob[b]
        # tidx16[chan, col] = take_idx[16*col + chan], replicated to all 8
        # GPSIMD core groups
        t16 = take_idx[b].astype(np.int16).reshape(C // QW, QW).T
        in_maps.append({
            "ident": ident, "identbf": identbf, "uti": uti,
            "hidden_t": np.ascontiguousarray(hidden_states[b].T),
            # maskw[q, j] = mask[16j + q]
            "maskw": np.ascontiguousarray(
                boundary_mask[b].astype(np.uint8).reshape(FW, QW).T),
            "probp": probp,
            "tidx16": np.ascontiguousarray(np.tile(t16, (P // QW, 1))),
        })
    return in_maps


last_results = None  # populated by kernel() for profiling harnesses


def kernel(hidden_states, boundary_mask, boundary_prob, take_idx,
           **run_kwargs) -> np.ndarray:
    global _CACHED_NC, last_results
    if _CACHED_NC is None:
        _CACHED_NC = build_nc()
    in_maps = _shard_inputs(hidden_states, boundary_mask, boundary_prob, take_idx)
    res = run_bass_kernel_spmd(_CACHED_NC, in_maps, core_ids=list(range(B)),
                               **run_kwargs)
    last_results = res
    outs = []
    for b in range(B):
        x = np.asarray(res.results[b]["out"])      # [128, 64, 512] bf16
        outs.append(x.transpose(1, 0, 2).reshape(L, D))
    return np.stack(outs, axis=0).astype(np.float32)


# revision 16
# speedup vs baseline: 1.2094x; 1.2094x over previous
"""Trainium2 Bass kernel for nn_DeChunkLayer (ragged_sequence).

Math (per batch row):
    p[c]     = clip(boundary_prob[take_idx[c]], EPS, 1-EPS)
    decay    = 1 - p, decay[0] = 0
    weighted = p * hidden, weighted[0] = hidden[0]
    smoothed[c] = decay[c] * smoothed[c-1] + weighted[c]      (EMA scan over C)
    chunk_id = clip(cumsum(boundary_mask) - 1, 0, C-1)
    out[l]   = smoothed[chunk_id[l]]

v5 vs v3 (baseline):
  - Output is bf16 in a partition-major DRAM layout [128, 64, D] (token
    l at [l%128, l//128]): every out-write descriptor covers 8KB
    contiguous DRAM (vs 2KB strided), and the bf16->f32 upconvert stage
    is gone entirely -- the host upcasts/unpermutes. Phase 3 is pure DMA.
  - smoothed DRAM scratch is p-major so the writeback is one contiguous
    16KB run per partition; gather rows are ck' = (c%128)*16 + c//128.
  - The p gather runs as 2 DMAGatherAnt calls over a host-padded
    [L, 64] prob table (elem=256B): no INDIRECT1D ucode, so GpSimd pays
    the gather-ant library load once (a tiny warmup call hides it) and
    the ~9us library-reload stall before the output-gather preps is gone.
  - The EMA scan runs in c-half-major order with a carry fold, so half A
    of smoothed lands early and gather call 0 (tokens < 1024 only
    reference chunks < 1024, since chunk_id[l] <= l) fires early.
  - p broadcast: 16 K=1 matmuls straight from the transposed p row (one
    partition each) -- no DRAM bounce.
  - GpSimd stream: [warmup, p-gather x2, preps 0-3, T(1)@wbA, prep4,
    T(3)@wbB, prep5, T(1), prep6, T(1), prep7, T(2)] so desc-gen never
    idles and each gather fires as soon as its prep + data are ready.
Tile does not emit RAW waits through DRAM scratch tensors, so the
triggers carry explicit sync deps on the smoothed writeback DMAs.
"""

import numpy as np

import concourse.bass as bass
import concourse.bacc as bacc
import concourse.mybir as mybir
import concourse.tile as tile
from concourse.bass_utils import run_bass_kernel_spmd

B, L, C, D = 8, 8192, 2048, 512
EPS = 1e-4
P = 128
NDG = D // P          # 4 partition groups of the transposed layout
NCB = C // P          # 16 c-blocks of 128
QW = 16               # wrap width of the dma_gather index layout
FW = L // QW          # 512 free positions in the wrapped layout
NGC = 8               # output dma_gather calls (num_idxs=1024 is a HW limit)
IPC = L // NGC        # 1024 indices per call
CH = C // 2           # half width of the scan split
PE = 64               # padded prob row: 64 f32 = 256B (min gather elem)

F32 = mybir.dt.float32
BF16 = mybir.dt.bfloat16
I16 = mybir.dt.int16
I32 = mybir.dt.int32
U8 = mybir.dt.uint8
COPY = mybir.ActivationFunctionType.Copy

_CACHED_NC = None


def build_nc() -> bacc.Bacc:
    nc = bacc.Bacc("TRN2", target_bir_lowering=False, debug=False)

    hidden_t = nc.dram_tensor("hidden_t", [D, C], F32, kind="ExternalInput")
    maskw_d = nc.dram_tensor("maskw", [QW, FW], U8, kind="ExternalInput")
    probp_d = nc.dram_tensor("probp", [L, PE], F32, kind="ExternalInput")
    tidx16_d = nc.dram_tensor("tidx16", [P, C // QW], I16, kind="ExternalInput")
    ident_d = nc.dram_tensor("ident", [P, P], F32, kind="ExternalInput")
    identbf_d = nc.dram_tensor("identbf", [P, P], BF16, kind="ExternalInput")
    uti_d = nc.dram_tensor("uti", [QW, QW], F32, kind="ExternalInput")
    out_d = nc.dram_tensor("out", [P, L // P, D], BF16, kind="ExternalOutput")

    with tile.TileContext(nc) as tc:
        with (
            tc.tile_pool(name="persist", bufs=1) as pp,
            tc.tile_pool(name="gather", bufs=NGC) as gp,
            tc.tile_pool(name="psum", bufs=2, space="PSUM") as psp,
        ):
            # psone (ck16 + p-row PSUM) closes before the 4-bank pb tile
            # is allocated (8 PSUM banks total).
            psone_cm = tc.tile_pool(name="psone", bufs=1, space="PSUM")
            psone = psone_cm.__enter__()
            # manual semaphores: one per output gather (the 16 per-queue
            # completion incs of call k don't distinguish calls on a shared
            # counter) plus one per p-gather call. alloc does not clear, so
            # clear them up front on the (idle) sync engine.
            dma_sems = [nc.alloc_semaphore(f"gdma{k}") for k in range(NGC)]
            psems = [nc.alloc_semaphore("pdma0")]
            for s in dma_sems + psems:
                nc.sync.sem_clear(s)

            # ---------------- input loads ----------------
            tidx16_sb = pp.tile([P, C // QW], I16)
            nc.sync.dma_start(tidx16_sb[:], tidx16_d[:])
            maskw_sb = pp.tile([QW, FW], U8)
            nc.scalar.dma_start(maskw_sb[:], maskw_d[:])

            # constants from the host (keeps GpSimd free for desc-gen)
            ident = pp.tile([P, P], F32)
            nc.sync.dma_start(ident[:], ident_d[:])
            ident_bf = pp.tile([P, P], BF16)
            nc.sync.dma_start(ident_bf[:], identbf_d[:])
            uti = pp.tile([QW, QW], F32)
            nc.sync.dma_start(uti[:], uti_d[:])

            # hidden loads issued up front (queues are idle in phase 1)
            hts = []
            for dg in range(NDG):
                ht = pp.tile([P, C], F32, tag=f"ht{dg}")
                nc.sync.dma_start(ht[:], hidden_t[dg * P:(dg + 1) * P, :])
                hts.append(ht)

            # ---------------- gpsimd: p gather (DMAGatherAnt) ----------
            # p2big[p, j, 0] = prob[take_idx[j*128 + p]] (elem pad of 64)
            p2big = pp.tile([P, NCB, PE], F32)
            pg0 = nc.gpsimd.dma_gather(
                out_ap=p2big[:, 0:NCB // 2, :], in_ap=probp_d[:],
                idxs_ap=tidx16_sb[:, 0:C // 2 // QW],
                num_idxs=C // 2, num_idxs_reg=C // 2, elem_size=PE,
            )
            pg0.then_inc(psems[0], 16)
            pg1 = nc.gpsimd.dma_gather(
                out_ap=p2big[:, NCB // 2:NCB, :], in_ap=probp_d[:],
                idxs_ap=tidx16_sb[:, C // 2 // QW:C // QW],
                num_idxs=C // 2, num_idxs_reg=C // 2, elem_size=PE,
            )
            pg1.then_inc(psems[0], 16)

            ones16 = pp.tile([1, QW], F32)
            nc.vector.memset(ones16[:], 1.0)
            ones161 = pp.tile([QW, 1], F32)
            nc.vector.memset(ones161[:], 1.0)
            ones1 = pp.tile([1, FW], F32)
            nc.vector.memset(ones1[:], 1.0)

            # ---------------- chunk ids (early: preps need ckrep) ----------
            # CK16[q, j] = chunk_id[16j + q]. Host passes maskw[q, j] =
            # mask[16j + q]. Within-column inclusive prefix over q via the
            # 16x16 inclusive triangular matmul; across-column exclusive
            # prefix of the column sums via a 1-partition scan, broadcast
            # into the same PSUM bank with a ones matmul.
            maskwf = pp.tile([QW, FW], F32)
            nc.vector.tensor_copy(maskwf[:], maskw_sb[:])
            ps16 = psone.tile([QW, FW], F32, space="PSUM", tag="ps16")
            nc.tensor.matmul(ps16[:], lhsT=uti[:], rhs=maskwf[:],
                             start=True, stop=False)
            cs_ps = psone.tile([1, FW], F32, space="PSUM", tag="cs")
            nc.tensor.matmul(cs_ps[:], lhsT=ones161[:], rhs=maskwf[:],
                             start=True, stop=True)
            colsb = pp.tile([1, FW], F32)
            nc.vector.tensor_copy(colsb[:], cs_ps[:])
            exc0 = pp.tile([1, FW], F32)
            nc.vector.tensor_tensor_scan(
                exc0[:], ones1[:], colsb[:],
                0.0, mybir.AluOpType.mult, mybir.AluOpType.add)
            nc.vector.tensor_tensor(exc0[:], exc0[:], colsb[:],
                                    mybir.AluOpType.subtract)
            nc.tensor.matmul(ps16[:], lhsT=ones16[:], rhs=exc0[:],
                             start=False, stop=True)
            ck = pp.tile([QW, FW], F32)
            nc.vector.tensor_scalar(ck[:], ps16[:], -1.0, None,
                                    mybir.AluOpType.add)
            nc.vector.tensor_scalar(ck[:], ck[:], 0.0, float(C - 1),
                                    mybir.AluOpType.max, mybir.AluOpType.min)
            # remap to the p-major smoothed layout: chunk c lives at row
            # (c%128)*16 + c//128 of the [2048, D] scratch view.
            cki = pp.tile([QW, FW], I32)
            nc.vector.tensor_copy(cki[:], ck[:])
            chi = pp.tile([QW, FW], I32)
            nc.vector.tensor_scalar(chi[:], cki[:], 7, None,
                                    mybir.AluOpType.logical_shift_right)
            nc.vector.tensor_scalar(cki[:], cki[:], 127, None,
                                    mybir.AluOpType.bitwise_and)
            nc.vector.tensor_scalar(cki[:], cki[:], 4, None,
                                    mybir.AluOpType.logical_shift_left)
            nc.vector.tensor_tensor(cki[:], cki[:], chi[:],
                                    mybir.AluOpType.add)
            ck16 = pp.tile([QW, FW], I16)
            nc.vector.tensor_copy(ck16[:], cki[:])
            # replicate to all 8 GPSIMD core groups (cross-partition copies)
            ckrep = pp.tile([P, FW], I16)
            for cgrp in range(P // QW):
                nc.scalar.dma_start(ckrep[cgrp * QW:(cgrp + 1) * QW, :], ck16[:])

            # prepared output gathers: desc-gen on GpSimd right after the p
            # gather (only needs ckrep); the DMAs fire at the triggers below.
            # sm_scratch is a raw (untracked) DRAM tensor so the preps do not
            # inherit the smoothed-writeback RAW dep -- that dep is carried
            # manually by the triggers. Layout is p-major: [p, ci, d].
            sm_dram = nc.dram_tensor("sm_scratch", [P, NCB, D], BF16,
                                     kind="Internal")
            sm_flat = sm_dram[:].rearrange("p ci d -> (p ci) d")
            gs = []

            def emit_prep(k):
                g = gp.tile([P, IPC // P, D], BF16, tag="g")
                nc.gpsimd.dma_gather(
                    out_ap=g[:], in_ap=sm_flat,
                    idxs_ap=ckrep[:, k * (FW // NGC):(k + 1) * (FW // NGC)],
                    num_idxs=IPC, num_idxs_reg=IPC, elem_size=D,
                    prepare_only=True, sem=dma_sems[k])
                gs.append(g)

            # GpSimd stream order == emission order: the first NEARLY preps
            # run while the scan pipeline computes; the remaining preps are
            # emitted between triggers so desc-gen overlaps phase-3 DMA
            # instead of stalling behind writeback sem waits.
            NEARLY = 4
            for k in range(NEARLY):
                emit_prep(k)

            # ---------------- p row -> PSUM broadcast ----------------
            p2 = p2big[:, :, 0:1].rearrange("p j o -> p (j o)")  # [128, 16]
            clip = nc.vector.tensor_scalar(p2, p2, EPS, 1.0 - EPS,
                                           mybir.AluOpType.max,
                                           mybir.AluOpType.min)
            clip._wait_ge(psems[0], 32)   # both p-gather calls landed
            nc.vector.memset(p2big[0:1, 0:1, 0:1], 1.0)  # w[0]=h[0], d[0]=0
            pT_ps = psone.tile([NCB, P], F32, space="PSUM", tag="prow")
            nc.tensor.transpose(pT_ps[:], p2, ident[:])
            pT = pp.tile([NCB, P], F32)
            nc.scalar.copy(pT[:], pT_ps[:])
            prow = pp.tile([1, C], F32)
            p_dram = nc.dram_tensor("p_row_scratch", [1, C], F32, kind="Internal")
            w_pr = nc.scalar.dma_start(
                p_dram[:].rearrange("o (j q) -> (o j) q", j=NCB), pT[:])
            r_pr = nc.scalar.dma_start(prow[:], p_dram[:])
            bass._add_dep_helper(r_pr.ins, w_pr.ins, sync=True,
                                 reason="p row bounce raw")
            ones128 = pp.tile([1, P], F32)
            nc.vector.memset(ones128[:], 1.0)
            psone_cm.__exit__(None, None, None)
            pbp_cm = tc.tile_pool(name="pbh", bufs=1, space="PSUM")
            pbp = pbp_cm.__enter__()
            pb_ps = pbp.tile([P, C], F32, space="PSUM", tag="pb")
            for n in range(C // 512):
                nc.tensor.matmul(pb_ps[:, n * 512:(n + 1) * 512],
                                 lhsT=ones128[:],
                                 rhs=prow[:, n * 512:(n + 1) * 512],
                                 start=True, stop=True)
            db = pp.tile([P, C], BF16)
            nc.scalar.activation(db[:], pb_ps[:], COPY, bias=1.0, scale=-1.0)

            # ---------------- EMA scan, c-half-major with carry fold -------
            # half A (chunks < 1024) completes for all 4 d-groups first so
            # its writeback (and gather call 0) can fire early.
            sm_sb = pp.tile([P, NCB * D], BF16)  # [c-in-block, (c-block, d)]
            wts, sts = [], []
            for dg in range(NDG):
                wt = pp.tile([P, C], BF16, tag=f"wt{dg}")
                st = pp.tile([P, C], BF16, tag=f"st{dg}")
                wts.append(wt)
                sts.append(st)

            def transpose_out(dg, ci, engine):
                ps = psp.tile([P, P], BF16, space="PSUM", tag="tps")
                nc.tensor.transpose(ps[:], sts[dg][:, ci * P:(ci + 1) * P],
                                    ident_bf[:])
                dst = sm_sb[:, ci * D + dg * P: ci * D + (dg + 1) * P]
                engine(dst, ps[:])

            for half in range(2):
                lo, hi = half * CH, (half + 1) * CH
                for dg in range(NDG):
                    wt, st = wts[dg], sts[dg]
                    if half == 0:
                        nc.vector.tensor_tensor(wt[:], hts[dg][:], pb_ps[:],
                                                mybir.AluOpType.mult)
                    else:
                        # fold the half-A carry into the first weighted col
                        carry = pp.tile([P, 1], F32, tag=f"cr{dg}")
                        nc.vector.tensor_tensor(
                            carry[:], st[:, CH - 1:CH], db[:, CH:CH + 1],
                            mybir.AluOpType.mult)
                        nc.vector.tensor_tensor(
                            wt[:, CH:CH + 1], wt[:, CH:CH + 1], carry[:],
                            mybir.AluOpType.add)
                    nc.vector.tensor_tensor_scan(
                        st[:, lo:hi], db[:, lo:hi], wt[:, lo:hi], 0.0,
                        mybir.AluOpType.mult, mybir.AluOpType.add)
                    # transpose this (half, dg)'s 8 c-blocks while the next
                    # scan runs; copies go to Scalar (DVE keeps scanning),
                    # split with DVE for the last d-group of each half so
                    # the half's writeback isn't gated on a scalar backlog.
                    for ci in range(half * (NCB // 2), (half + 1) * (NCB // 2)):
                        if dg == NDG - 1 and ci % 2 == 0:
                            transpose_out(dg, ci, nc.vector.tensor_copy)
                        else:
                            transpose_out(dg, ci, nc.scalar.copy)
                # writeback this half: 2D flat APs -> one contiguous 8KB run
                # per partition on both sides (big descriptors)
                hb, he = half * (NCB // 2), (half + 1) * (NCB // 2)
                w = nc.sync.dma_start(
                    sm_dram[:].rearrange("p ci d -> p (ci d)")[:, hb * D:he * D],
                    sm_sb[:, hb * D:he * D])
                if half == 0:
                    w_sm_a = w
                else:
                    w_sm_b = w

            # ---------------- output expansion ----------------
            # triggers fire each prepared gather once the smoothed data it
            # can reference has landed: call 0 only touches chunks < 1024
            # (chunk_id[l] <= l), later calls may touch anything.
            trig_of = []

            def emit_trigger(count, need_b):
                tr = nc.gpsimd.trigger_dma(count=count)
                bass._add_dep_helper(tr.ins, w_sm_a.ins, sync=True,
                                     reason="smoothed gather raw a")
                if need_b:
                    bass._add_dep_helper(tr.ins, w_sm_b.ins, sync=True,
                                         reason="smoothed gather raw b")
                trig_of.extend([tr] * count)
                return tr

            emit_trigger(1, need_b=False)   # call 0
            emit_prep(4)
            emit_trigger(3, need_b=True)    # calls 1-3
            emit_prep(5)
            emit_trigger(1, need_b=True)    # call 4
            emit_prep(6)
            emit_trigger(1, need_b=True)    # call 5
            emit_prep(7)
            emit_trigger(2, need_b=True)    # calls 6-7

            for k in range(NGC):
                # Tile's auto DMASW wait is satisfied by the prep-time
                # pre-bump, not the gather's completion; the baked per-call
                # sem is the real data-ready signal. The no-sync edge on the
                # trigger keeps the scheduler from hoisting this wait ahead
                # of the phase-1 work on its engine.
                # 2D [128, 4096] APs on both sides: one contiguous 8KB run
                # per partition, so the AP normalizer emits few big
                # descriptors instead of 1024 per-(p,g) 1KB ones.
                GW = (IPC // P) * D
                od = nc.sync.dma_start(
                    out_d[:].rearrange("p g d -> p (g d)")[:, k * GW:(k + 1) * GW],
                    gs[k][:].rearrange("p g d -> p (g d)"))
                od._wait_ge(dma_sems[k], 16)
                bass._add_dep_helper(od.ins, trig_of[k].ins, sync=False,
                                     reason="out write after trigger")
            pbp_cm.__exit__(None, None, None)

    nc.compile()
    return nc


def _shard_inputs(hidden_states, boundary_mask, boundary_prob, take_idx):
    import ml_dtypes
    hidden_states = np.asarray(hidden_states, dtype=np.float32)
    boundary_mask = np.asarray(boundary_mask)
    boundary_prob = np.asarray(boundary_prob, dtype=np.float32)
    take_idx = np.asarray(take_idx)
    ident = np.eye(P, dtype=np.float32)
    identbf = np.eye(P).astype(ml_dtypes.bfloat16)
    uti = np.triu(np.ones((QW, QW), dtype=np.float32))
    in_maps = []
    for b in range(B):
        probp = np.zeros((L, PE), dtype=np.float32)
        probp[:, 0] = boundary_prob[b]
        # tidx16[chan, col] = take_idx[16*col + chan], replicated to all 8
        # GPSIMD core groups
        t16 = take_idx[b].astype(np.int16).reshape(C // QW, QW).T
        in_maps.append({
            "ident": ident, "identbf": identbf, "uti": uti,
            "hidden_t": np.ascontiguousarray(hidden_states[b].T),
            # maskw[q, j] = mask[16j + q]
            "maskw": np.ascontiguousarray(
                boundary_mask[b].astype(np.uint8).reshape(FW, QW).T),
            "probp": probp,
            "tidx16": np.ascontiguousarray(np.tile(t16, (P // QW, 1))),
        })
    return in_maps


last_results = None  # populated by kernel() for profiling harnesses


def kernel(hidden_states, boundary_mask, boundary_prob, take_idx,
           **run_kwargs) -> np.ndarray:
    global _CACHED_NC, last_results
    if _CACHED_NC is None:
        _CACHED_NC = build_nc()
    in_maps = _shard_inputs(hidden_states, boundary_mask, boundary_prob, take_idx)
    res = run_bass_kernel_spmd(_CACHED_NC, in_maps, core_ids=list(range(B)),
                               **run_kwargs)
    last_results = res
    outs = []
    for b in range(B):
        x = np.asarray(res.results[b]["out"])      # [128, 64, 512] bf16
        outs.append(x.transpose(1, 0, 2).reshape(L, D))
    return np.stack(outs, axis=0).astype(np.float32)


# revision 20
# speedup vs baseline: 1.2212x; 1.0097x over previous
"""Trainium2 Bass kernel for nn_DeChunkLayer (ragged_sequence).

Math (per batch row):
    p[c]     = clip(boundary_prob[take_idx[c]], EPS, 1-EPS)
    decay    = 1 - p, decay[0] = 0
    weighted = p * hidden, weighted[0] = hidden[0]
    smoothed[c] = decay[c] * smoothed[c-1] + weighted[c]      (EMA scan over C)
    chunk_id = clip(cumsum(boundary_mask) - 1, 0, C-1)
    out[l]   = smoothed[chunk_id[l]]

v5 vs v3 (baseline):
  - Output is bf16 in a partition-major DRAM layout [128, 64, D] (token
    l at [l%128, l//128]): every out-write descriptor covers 8KB
    contiguous DRAM (vs 2KB strided), and the bf16->f32 upconvert stage
    is gone entirely -- the host upcasts/unpermutes. Phase 3 is pure DMA.
  - smoothed DRAM scratch is p-major so the writeback is one contiguous
    16KB run per partition; gather rows are ck' = (c%128)*16 + c//128.
  - The p gather runs as 2 DMAGatherAnt calls over a host-padded
    [L, 64] prob table (elem=256B): no INDIRECT1D ucode, so GpSimd pays
    the gather-ant library load once (a tiny warmup call hides it) and
    the ~9us library-reload stall before the output-gather preps is gone.
  - The EMA scan runs in c-half-major order with a carry fold, so half A
    of smoothed lands early and gather call 0 (tokens < 1024 only
    reference chunks < 1024, since chunk_id[l] <= l) fires early.
  - p broadcast: 16 K=1 matmuls straight from the transposed p row (one
    partition each) -- no DRAM bounce.
  - GpSimd stream: [warmup, p-gather x2, preps 0-3, T(1)@wbA, prep4,
    T(3)@wbB, prep5, T(1), prep6, T(1), prep7, T(2)] so desc-gen never
    idles and each gather fires as soon as its prep + data are ready.
Tile does not emit RAW waits through DRAM scratch tensors, so the
triggers carry explicit sync deps on the smoothed writeback DMAs.
"""

import numpy as np

import concourse.bass as bass
import concourse.bacc as bacc
import concourse.mybir as mybir
import concourse.tile as tile
from concourse.bass_utils import run_bass_kernel_spmd

B, L, C, D = 8, 8192, 2048, 512
EPS = 1e-4
P = 128
NDG = D // P          # 4 partition groups of the transposed layout
NCB = C // P          # 16 c-blocks of 128
QW = 16               # wrap width of the dma_gather index layout
FW = L // QW          # 512 free positions in the wrapped layout
NGC = 8               # output dma_gather calls (num_idxs=1024 is a HW limit)
IPC = L // NGC        # 1024 indices per call
CH = C // 2           # half width of the scan split
PE = 64               # padded prob row: 64 f32 = 256B (min gather elem)

F32 = mybir.dt.float32
BF16 = mybir.dt.bfloat16
I16 = mybir.dt.int16
I32 = mybir.dt.int32
U8 = mybir.dt.uint8
COPY = mybir.ActivationFunctionType.Copy

_CACHED_NC = None


def build_nc() -> bacc.Bacc:
    nc = bacc.Bacc("TRN2", target_bir_lowering=False, debug=False)

    hidden_t = nc.dram_tensor("hidden_t", [D, C], F32, kind="ExternalInput")
    maskw_d = nc.dram_tensor("maskw", [QW, FW], U8, kind="ExternalInput")
    probp_d = nc.dram_tensor("probp", [L, PE], F32, kind="ExternalInput")
    tidx16_d = nc.dram_tensor("tidx16", [P, C // QW], I16, kind="ExternalInput")
    ident_d = nc.dram_tensor("ident", [P, P], F32, kind="ExternalInput")
    identbf_d = nc.dram_tensor("identbf", [P, P], BF16, kind="ExternalInput")
    uti_d = nc.dram_tensor("uti", [QW, QW], F32, kind="ExternalInput")
    out_d = nc.dram_tensor("out", [P, L // P, D], BF16, kind="ExternalOutput")

    with tile.TileContext(nc) as tc:
        with (
            tc.tile_pool(name="persist", bufs=1) as pp,
            tc.tile_pool(name="gather", bufs=NGC) as gp,
            tc.tile_pool(name="psum", bufs=2, space="PSUM") as psp,
        ):
            # psone (ck16 + p-row PSUM) closes before the 4-bank pb tile
            # is allocated (8 PSUM banks total).
            psone_cm = tc.tile_pool(name="psone", bufs=1, space="PSUM")
            psone = psone_cm.__enter__()
            # manual semaphores: one per output gather (the 16 per-queue
            # completion incs of call k don't distinguish calls on a shared
            # counter) plus one per p-gather call. alloc does not clear, so
            # clear them up front on the (idle) sync engine.
            dma_sems = [nc.alloc_semaphore(f"gdma{k}") for k in range(NGC)]
            psems = [nc.alloc_semaphore("pdma0")]
            for s in dma_sems + psems:
                nc.sync.sem_clear(s)

            # ---------------- input loads ----------------
            tidx16_sb = pp.tile([P, C // QW], I16)
            nc.sync.dma_start(tidx16_sb[:], tidx16_d[:])
            maskw_sb = pp.tile([QW, FW], U8)
            nc.scalar.dma_start(maskw_sb[:], maskw_d[:])

            # constants from the host (keeps GpSimd free for desc-gen)
            ident = pp.tile([P, P], F32)
            nc.sync.dma_start(ident[:], ident_d[:])
            ident_bf = pp.tile([P, P], BF16)
            nc.sync.dma_start(ident_bf[:], identbf_d[:])
            uti = pp.tile([QW, QW], F32)
            nc.sync.dma_start(uti[:], uti_d[:])

            # hidden loads issued up front (queues are idle in phase 1)
            hts = []
            for dg in range(NDG):
                ht = pp.tile([P, C], F32, tag=f"ht{dg}")
                nc.sync.dma_start(ht[:], hidden_t[dg * P:(dg + 1) * P, :])
                hts.append(ht)

            # ---------------- gpsimd: p gather (DMAGatherAnt) ----------
            # p2big[p, j, 0] = prob[take_idx[j*128 + p]] (elem pad of 64)
            p2big = pp.tile([P, NCB, PE], F32)
            pg0 = nc.gpsimd.dma_gather(
                out_ap=p2big[:, 0:NCB // 2, :], in_ap=probp_d[:],
                idxs_ap=tidx16_sb[:, 0:C // 2 // QW],
                num_idxs=C // 2, num_idxs_reg=C // 2, elem_size=PE,
            )
            pg0.then_inc(psems[0], 16)
            pg1 = nc.gpsimd.dma_gather(
                out_ap=p2big[:, NCB // 2:NCB, :], in_ap=probp_d[:],
                idxs_ap=tidx16_sb[:, C // 2 // QW:C // QW],
                num_idxs=C // 2, num_idxs_reg=C // 2, elem_size=PE,
            )
            pg1.then_inc(psems[0], 16)

            ones16 = pp.tile([1, QW], F32)
            nc.vector.memset(ones16[:], 1.0)
            ones161 = pp.tile([QW, 1], F32)
            nc.vector.memset(ones161[:], 1.0)
            ones1 = pp.tile([1, FW], F32)
            nc.vector.memset(ones1[:], 1.0)

            # ---------------- chunk ids (early: preps need ckrep) ----------
            # CK16[q, j] = chunk_id[16j + q]. Host passes maskw[q, j] =
            # mask[16j + q]. Within-column inclusive prefix over q via the
            # 16x16 inclusive triangular matmul; across-column exclusive
            # prefix of the column sums via a 1-partition scan, broadcast
            # into the same PSUM bank with a ones matmul.
            maskwf = pp.tile([QW, FW], F32)
            nc.vector.tensor_copy(maskwf[:], maskw_sb[:])
            ps16 = psone.tile([QW, FW], F32, space="PSUM", tag="ps16")
            nc.tensor.matmul(ps16[:], lhsT=uti[:], rhs=maskwf[:],
                             start=True, stop=False)
            cs_ps = psone.tile([1, FW], F32, space="PSUM", tag="cs")
            nc.tensor.matmul(cs_ps[:], lhsT=ones161[:], rhs=maskwf[:],
                             start=True, stop=True)
            colsb = pp.tile([1, FW], F32)
            nc.vector.tensor_copy(colsb[:], cs_ps[:])
            exc0 = pp.tile([1, FW], F32)
            nc.vector.tensor_tensor_scan(
                exc0[:], ones1[:], colsb[:],
                0.0, mybir.AluOpType.mult, mybir.AluOpType.add)
            nc.vector.tensor_tensor(exc0[:], exc0[:], colsb[:],
                                    mybir.AluOpType.subtract)
            nc.tensor.matmul(ps16[:], lhsT=ones16[:], rhs=exc0[:],
                             start=False, stop=True)
            ck = pp.tile([QW, FW], F32)
            nc.vector.tensor_scalar(ck[:], ps16[:], -1.0, None,
                                    mybir.AluOpType.add)
            nc.vector.tensor_scalar(ck[:], ck[:], 0.0, float(C - 1),
                                    mybir.AluOpType.max, mybir.AluOpType.min)
            # remap to the p-major smoothed layout: chunk c lives at row
            # (c%128)*16 + c//128 of the [2048, D] scratch view.
            cki = pp.tile([QW, FW], I32)
            nc.vector.tensor_copy(cki[:], ck[:])
            chi = pp.tile([QW, FW], I32)
            nc.vector.tensor_scalar(chi[:], cki[:], 7, None,
                                    mybir.AluOpType.logical_shift_right)
            nc.vector.tensor_scalar(cki[:], cki[:], 127, None,
                                    mybir.AluOpType.bitwise_and)
            nc.vector.tensor_scalar(cki[:], cki[:], 4, None,
                                    mybir.AluOpType.logical_shift_left)
            nc.vector.tensor_tensor(cki[:], cki[:], chi[:],
                                    mybir.AluOpType.add)
            ck16 = pp.tile([QW, FW], I16)
            ck16_cp = nc.vector.tensor_copy(ck16[:], cki[:])
            # replicate to all 8 GPSIMD core groups (cross-partition copies)
            ckrep = pp.tile([P, FW], I16)
            for cgrp in range(P // QW):
                nc.scalar.dma_start(ckrep[cgrp * QW:(cgrp + 1) * QW, :], ck16[:])

            # prepared output gathers: desc-gen on GpSimd right after the p
            # gather (only needs ckrep); the DMAs fire at the triggers below.
            # sm_scratch is a raw (untracked) DRAM tensor so the preps do not
            # inherit the smoothed-writeback RAW dep -- that dep is carried
            # manually by the triggers. Layout is p-major: [p, ci, d].
            sm_dram = nc.dram_tensor("sm_scratch", [P, NCB, D], BF16,
                                     kind="Internal")
            sm_flat = sm_dram[:].rearrange("p ci d -> (p ci) d")
            gs = []

            def emit_prep(k):
                g = gp.tile([P, IPC // P, D], BF16, tag="g")
                nc.gpsimd.dma_gather(
                    out_ap=g[:], in_ap=sm_flat,
                    idxs_ap=ckrep[:, k * (FW // NGC):(k + 1) * (FW // NGC)],
                    num_idxs=IPC, num_idxs_reg=IPC, elem_size=D,
                    prepare_only=True, sem=dma_sems[k])
                gs.append(g)

            # GpSimd stream order == emission order: the first NEARLY preps
            # run while the scan pipeline computes; the remaining preps are
            # emitted between triggers so desc-gen overlaps phase-3 DMA
            # instead of stalling behind writeback sem waits.
            NEARLY = 4
            for k in range(NEARLY):
                emit_prep(k)

            # ---------------- p row -> PSUM broadcast ----------------
            p2 = p2big[:, :, 0:1].rearrange("p j o -> p (j o)")  # [128, 16]
            clip = nc.vector.tensor_scalar(p2, p2, EPS, 1.0 - EPS,
                                           mybir.AluOpType.max,
                                           mybir.AluOpType.min)
            clip._wait_ge(psems[0], 32)   # both p-gather calls landed
            # no-sync edge: clip's sem wait stalls the DVE stream until the
            # p-gather data lands, so keep Tile from hoisting it ahead of
            # the chunk-id pipeline (which gates the gather preps).
            bass._add_dep_helper(clip.ins, ck16_cp.ins, sync=False,
                                 reason="clip after ck pipeline")
            nc.vector.memset(p2big[0:1, 0:1, 0:1], 1.0)  # w[0]=h[0], d[0]=0
            pT_ps = psone.tile([NCB, P], F32, space="PSUM", tag="prow")
            nc.tensor.transpose(pT_ps[:], p2, ident[:])
            pT = pp.tile([NCB, P], F32)
            nc.scalar.copy(pT[:], pT_ps[:])
            prow = pp.tile([1, C], F32)
            p_dram = nc.dram_tensor("p_row_scratch", [1, C], F32, kind="Internal")
            w_pr = nc.scalar.dma_start(
                p_dram[:].rearrange("o (j q) -> (o j) q", j=NCB), pT[:])
            r_pr = nc.scalar.dma_start(prow[:], p_dram[:])
            bass._add_dep_helper(r_pr.ins, w_pr.ins, sync=True,
                                 reason="p row bounce raw")
            ones128 = pp.tile([1, P], F32)
            nc.vector.memset(ones128[:], 1.0)
            psone_cm.__exit__(None, None, None)
            pbp_cm = tc.tile_pool(name="pbh", bufs=1, space="PSUM")
            pbp = pbp_cm.__enter__()
            pb_ps = pbp.tile([P, C], F32, space="PSUM", tag="pb")
            for n in range(C // 512):
                nc.tensor.matmul(pb_ps[:, n * 512:(n + 1) * 512],
                                 lhsT=ones128[:],
                                 rhs=prow[:, n * 512:(n + 1) * 512],
                                 start=True, stop=True)
            db = pp.tile([P, C], BF16)
            nc.scalar.activation(db[:], pb_ps[:], COPY, bias=1.0, scale=-1.0)

            # ---------------- EMA scan, c-half-major with carry fold -------
            # half A (chunks < 1024) completes for all 4 d-groups first so
            # its writeback (and gather call 0) can fire early.
            sm_sb = pp.tile([P, NCB * D], BF16)  # [c-in-block, (c-block, d)]
            wts, sts = [], []
            for dg in range(NDG):
                wt = pp.tile([P, C], BF16, tag=f"wt{dg}")
                st = pp.tile([P, C], BF16, tag=f"st{dg}")
                wts.append(wt)
                sts.append(st)

            def transpose_out(dg, ci, engine):
                ps = psp.tile([P, P], BF16, space="PSUM", tag="tps")
                nc.tensor.transpose(ps[:], sts[dg][:, ci * P:(ci + 1) * P],
                                    ident_bf[:])
                dst = sm_sb[:, ci * D + dg * P: ci * D + (dg + 1) * P]
                engine(dst, ps[:])

            for half in range(2):
                lo, hi = half * CH, (half + 1) * CH
                for dg in range(NDG):
                    wt, st = wts[dg], sts[dg]
                    if half == 0:
                        nc.vector.tensor_tensor(wt[:], hts[dg][:], pb_ps[:],
                                                mybir.AluOpType.mult)
                    else:
                        # fold the half-A carry into the first weighted col
                        carry = pp.tile([P, 1], F32, tag=f"cr{dg}")
                        nc.vector.tensor_tensor(
                            carry[:], st[:, CH - 1:CH], db[:, CH:CH + 1],
                            mybir.AluOpType.mult)
                        nc.vector.tensor_tensor(
                            wt[:, CH:CH + 1], wt[:, CH:CH + 1], carry[:],
                            mybir.AluOpType.add)
                    nc.vector.tensor_tensor_scan(
                        st[:, lo:hi], db[:, lo:hi], wt[:, lo:hi], 0.0,
                        mybir.AluOpType.mult, mybir.AluOpType.add)
                    # transpose this (half, dg)'s 8 c-blocks while the next
                    # scan runs; copies go to Scalar (DVE keeps scanning),
                    # except half 1's last d-group splits onto DVE (free
                    # after the final scan) so wbB isn't scalar-gated.
                    for ci in range(half * (NCB // 2), (half + 1) * (NCB // 2)):
                        if half == 1 and dg == NDG - 1 and ci % 2 == 0:
                            transpose_out(dg, ci, nc.vector.tensor_copy)
                        else:
                            transpose_out(dg, ci, nc.scalar.copy)
                # writeback this half: 2D flat APs -> one contiguous 8KB run
                # per partition on both sides (big descriptors)
                hb, he = half * (NCB // 2), (half + 1) * (NCB // 2)
                w = nc.sync.dma_start(
                    sm_dram[:].rearrange("p ci d -> p (ci d)")[:, hb * D:he * D],
                    sm_sb[:, hb * D:he * D])
                if half == 0:
                    w_sm_a = w
                else:
                    w_sm_b = w

            # ---------------- output expansion ----------------
            # triggers fire each prepared gather once the smoothed data it
            # can reference has landed: call 0 only touches chunks < 1024
            # (chunk_id[l] <= l), later calls may touch anything.
            trig_of = []

            def emit_trigger(count, need_b):
                tr = nc.gpsimd.trigger_dma(count=count)
                bass._add_dep_helper(tr.ins, w_sm_a.ins, sync=True,
                                     reason="smoothed gather raw a")
                if need_b:
                    bass._add_dep_helper(tr.ins, w_sm_b.ins, sync=True,
                                         reason="smoothed gather raw b")
                trig_of.extend([tr] * count)
                return tr

            emit_trigger(1, need_b=False)   # call 0
            emit_prep(4)
            emit_trigger(3, need_b=True)    # calls 1-3
            emit_prep(5)
            emit_trigger(1, need_b=True)    # call 4
            emit_prep(6)
            emit_trigger(1, need_b=True)    # call 5
            emit_prep(7)
            emit_trigger(2, need_b=True)    # calls 6-7

            for k in range(NGC):
                # Tile's auto DMASW wait is satisfied by the prep-time
                # pre-bump, not the gather's completion; the baked per-call
                # sem is the real data-ready signal. The no-sync edge on the
                # trigger keeps the scheduler from hoisting this wait ahead
                # of the phase-1 work on its engine.
                # 2D [128, 4096] APs on both sides: one contiguous 8KB run
                # per partition, so the AP normalizer emits few big
                # descriptors instead of 1024 per-(p,g) 1KB ones. Alternate
                # the two HWDGE rings (sync=SP, scalar=ACT) -- HWDGE DMAs
                # are FIFO per ring, so one ring would serialize all 8.
                GW = (IPC // P) * D
                eng = nc.sync if k % 2 == 0 else nc.scalar
                od = eng.dma_start(
                    out_d[:].rearrange("p g d -> p (g d)")[:, k * GW:(k + 1) * GW],
                    gs[k][:].rearrange("p g d -> p (g d)"))
                od._wait_ge(dma_sems[k], 16)
                bass._add_dep_helper(od.ins, trig_of[k].ins, sync=False,
                                     reason="out write after trigger")
            pbp_cm.__exit__(None, None, None)

    nc.compile()
    return nc


def _shard_inputs(hidden_states, boundary_mask, boundary_prob, take_idx):
    import ml_dtypes
    hidden_states = np.asarray(hidden_states, dtype=np.float32)
    boundary_mask = np.asarray(boundary_mask)
    boundary_prob = np.asarray(boundary_prob, dtype=np.float32)
    take_idx = np.asarray(take_idx)
    ident = np.eye(P, dtype=np.float32)
    identbf = np.eye(P).astype(ml_dtypes.bfloat16)
    uti = np.triu(np.ones((QW, QW), dtype=np.float32))
    in_maps = []
    for b in range(B):
        probp = np.zeros((L, PE), dtype=np.float32)
        probp[:, 0] = boundary_prob[b]
        # tidx16[chan, col] = take_idx[16*col + chan], replicated to all 8
        # GPSIMD core groups
        t16 = take_idx[b].astype(np.int16).reshape(C // QW, QW).T
        in_maps.append({
            "ident": ident, "identbf": identbf, "uti": uti,
            "hidden_t": np.ascontiguousarray(hidden_states[b].T),
            # maskw[q, j] = mask[16j + q]
            "maskw": np.ascontiguousarray(
                boundary_mask[b].astype(np.uint8).reshape(FW, QW).T),
            "probp": probp,
            "tidx16": np.ascontiguousarray(np.tile(t16, (P // QW, 1))),
        })
    return in_maps


last_results = None  # populated by kernel() for profiling harnesses


def kernel(hidden_states, boundary_mask, boundary_prob, take_idx,
           **run_kwargs) -> np.ndarray:
    global _CACHED_NC, last_results
    if _CACHED_NC is None:
        _CACHED_NC = build_nc()
    in_maps = _shard_inputs(hidden_states, boundary_mask, boundary_prob, take_idx)
    res = run_bass_kernel_spmd(_CACHED_NC, in_maps, core_ids=list(range(B)),
                               **run_kwargs)
    last_results = res
    outs = []
    for b in range(B):
        x = np.asarray(res.results[b]["out"])      # [128, 64, 512] bf16
        outs.append(x.transpose(1, 0, 2).reshape(L, D))
    return np.stack(outs, axis=0).astype(np.float32)


# revision 24
# speedup vs baseline: 1.2455x; 1.0199x over previous
"""Trainium2 Bass kernel for nn_DeChunkLayer (ragged_sequence).

Math (per batch row):
    p[c]     = clip(boundary_prob[take_idx[c]], EPS, 1-EPS)
    decay    = 1 - p, decay[0] = 0
    weighted = p * hidden, weighted[0] = hidden[0]
    smoothed[c] = decay[c] * smoothed[c-1] + weighted[c]      (EMA scan over C)
    chunk_id = clip(cumsum(boundary_mask) - 1, 0, C-1)
    out[l]   = smoothed[chunk_id[l]]

v5 vs v3 (baseline):
  - Output is bf16 in a partition-major DRAM layout [128, 64, D] (token
    l at [l%128, l//128]): every out-write descriptor covers 8KB
    contiguous DRAM (vs 2KB strided), and the bf16->f32 upconvert stage
    is gone entirely -- the host upcasts/unpermutes. Phase 3 is pure DMA.
  - smoothed DRAM scratch is p-major so the writeback is one contiguous
    16KB run per partition; gather rows are ck' = (c%128)*16 + c//128.
  - The p gather runs as 2 DMAGatherAnt calls over a host-padded
    [L, 64] prob table (elem=256B): no INDIRECT1D ucode, so GpSimd pays
    the gather-ant library load once (a tiny warmup call hides it) and
    the ~9us library-reload stall before the output-gather preps is gone.
  - The EMA scan runs in c-half-major order with a carry fold, so half A
    of smoothed lands early and gather call 0 (tokens < 1024 only
    reference chunks < 1024, since chunk_id[l] <= l) fires early.
  - p broadcast: 16 K=1 matmuls straight from the transposed p row (one
    partition each) -- no DRAM bounce.
  - GpSimd stream: [warmup, p-gather x2, preps 0-3, T(1)@wbA, prep4,
    T(3)@wbB, prep5, T(1), prep6, T(1), prep7, T(2)] so desc-gen never
    idles and each gather fires as soon as its prep + data are ready.
Tile does not emit RAW waits through DRAM scratch tensors, so the
triggers carry explicit sync deps on the smoothed writeback DMAs.
"""

import numpy as np

import concourse.bass as bass
import concourse.bacc as bacc
import concourse.mybir as mybir
import concourse.tile as tile
from concourse.bass_utils import run_bass_kernel_spmd

B, L, C, D = 8, 8192, 2048, 512
EPS = 1e-4
P = 128
NDG = D // P          # 4 partition groups of the transposed layout
NCB = C // P          # 16 c-blocks of 128
QW = 16               # wrap width of the dma_gather index layout
FW = L // QW          # 512 free positions in the wrapped layout
NGC = 8               # output dma_gather calls (num_idxs=1024 is a HW limit)
IPC = L // NGC        # 1024 indices per call
CH = C // 2           # half width of the scan split
PE = 64               # padded prob row: 64 f32 = 256B (min gather elem)

F32 = mybir.dt.float32
BF16 = mybir.dt.bfloat16
I16 = mybir.dt.int16
I32 = mybir.dt.int32
U8 = mybir.dt.uint8
COPY = mybir.ActivationFunctionType.Copy

_CACHED_NC = None


def build_nc() -> bacc.Bacc:
    nc = bacc.Bacc("TRN2", target_bir_lowering=False, debug=False)

    hidden_t = nc.dram_tensor("hidden_t", [D, C], F32, kind="ExternalInput")
    maskw_d = nc.dram_tensor("maskw", [QW, FW], U8, kind="ExternalInput")
    probp_d = nc.dram_tensor("probp", [L, PE], F32, kind="ExternalInput")
    tidx16_d = nc.dram_tensor("tidx16", [P, C // QW], I16, kind="ExternalInput")
    ident_d = nc.dram_tensor("ident", [P, P], F32, kind="ExternalInput")
    identbf_d = nc.dram_tensor("identbf", [P, P], BF16, kind="ExternalInput")
    uti_d = nc.dram_tensor("uti", [QW, QW], F32, kind="ExternalInput")
    out_d = nc.dram_tensor("out", [P, L // P, D], BF16, kind="ExternalOutput")

    with tile.TileContext(nc) as tc:
        with (
            tc.tile_pool(name="persist", bufs=1) as pp,
            tc.tile_pool(name="gather", bufs=NGC) as gp,
            tc.tile_pool(name="psum", bufs=2, space="PSUM") as psp,
        ):
            # psone (ck16 + p-row PSUM) closes before the 4-bank pb tile
            # is allocated (8 PSUM banks total).
            psone_cm = tc.tile_pool(name="psone", bufs=1, space="PSUM")
            psone = psone_cm.__enter__()
            # manual semaphores: one per output gather (the 16 per-queue
            # completion incs of call k don't distinguish calls on a shared
            # counter) plus one per p-gather call. alloc does not clear, so
            # clear them up front on the (idle) sync engine.
            dma_sems = [nc.alloc_semaphore(f"gdma{k}") for k in range(NGC)]
            psems = [nc.alloc_semaphore(f"pdma{j}") for j in range(2)]
            for s in dma_sems + psems:
                nc.sync.sem_clear(s)

            # ---------------- input loads ----------------
            tidx16_sb = pp.tile([P, C // QW], I16)
            nc.sync.dma_start(tidx16_sb[:], tidx16_d[:])
            maskw_sb = pp.tile([QW, FW], U8)
            nc.scalar.dma_start(maskw_sb[:], maskw_d[:])

            # constants from the host (keeps GpSimd free for desc-gen)
            ident = pp.tile([P, P], F32)
            nc.sync.dma_start(ident[:], ident_d[:])
            ident_bf = pp.tile([P, P], BF16)
            nc.sync.dma_start(ident_bf[:], identbf_d[:])
            uti = pp.tile([QW, QW], F32)
            nc.sync.dma_start(uti[:], uti_d[:])

            # hidden loads issued up front (queues are idle in phase 1)
            hts = []
            for dg in range(NDG):
                ht = pp.tile([P, C], F32, tag=f"ht{dg}")
                nc.sync.dma_start(ht[:], hidden_t[dg * P:(dg + 1) * P, :])
                hts.append(ht)

            # ---------------- gpsimd: p gather (DMAGatherAnt) ----------
            # p2big[p, j, 0] = prob[take_idx[j*128 + p]] (elem pad of 64)
            p2big = pp.tile([P, NCB, PE], F32)
            pg0 = nc.gpsimd.dma_gather(
                out_ap=p2big[:, 0:NCB // 2, :], in_ap=probp_d[:],
                idxs_ap=tidx16_sb[:, 0:C // 2 // QW],
                num_idxs=C // 2, num_idxs_reg=C // 2, elem_size=PE,
            )
            pg0.then_inc(psems[0], 16)
            pg1 = nc.gpsimd.dma_gather(
                out_ap=p2big[:, NCB // 2:NCB, :], in_ap=probp_d[:],
                idxs_ap=tidx16_sb[:, C // 2 // QW:C // QW],
                num_idxs=C // 2, num_idxs_reg=C // 2, elem_size=PE,
            )
            pg1.then_inc(psems[1], 16)

            ones16 = pp.tile([1, QW], F32)
            nc.vector.memset(ones16[:], 1.0)
            ones161 = pp.tile([QW, 1], F32)
            nc.vector.memset(ones161[:], 1.0)
            ones1 = pp.tile([1, FW], F32)
            nc.vector.memset(ones1[:], 1.0)

            # ---------------- chunk ids (early: preps need ckrep) ----------
            # CK16[q, j] = chunk_id[16j + q]. Host passes maskw[q, j] =
            # mask[16j + q]. Within-column inclusive prefix over q via the
            # 16x16 inclusive triangular matmul; across-column exclusive
            # prefix of the column sums via a 1-partition scan, broadcast
            # into the same PSUM bank with a ones matmul.
            maskwf = pp.tile([QW, FW], F32)
            nc.vector.tensor_copy(maskwf[:], maskw_sb[:])
            ps16 = psone.tile([QW, FW], F32, space="PSUM", tag="ps16")
            nc.tensor.matmul(ps16[:], lhsT=uti[:], rhs=maskwf[:],
                             start=True, stop=False)
            cs_ps = psone.tile([1, FW], F32, space="PSUM", tag="cs")
            nc.tensor.matmul(cs_ps[:], lhsT=ones161[:], rhs=maskwf[:],
                             start=True, stop=True)
            colsb = pp.tile([1, FW], F32)
            nc.vector.tensor_copy(colsb[:], cs_ps[:])
            exc0 = pp.tile([1, FW], F32)
            nc.vector.tensor_tensor_scan(
                exc0[:], ones1[:], colsb[:],
                0.0, mybir.AluOpType.mult, mybir.AluOpType.add)
            nc.vector.tensor_tensor(exc0[:], exc0[:], colsb[:],
                                    mybir.AluOpType.subtract)
            nc.tensor.matmul(ps16[:], lhsT=ones16[:], rhs=exc0[:],
                             start=False, stop=True)
            ck = pp.tile([QW, FW], F32)
            nc.vector.tensor_scalar(ck[:], ps16[:], -1.0, None,
                                    mybir.AluOpType.add)
            nc.vector.tensor_scalar(ck[:], ck[:], 0.0, float(C - 1),
                                    mybir.AluOpType.max, mybir.AluOpType.min)
            # remap to the p-major smoothed layout: chunk c lives at row
            # (c%128)*16 + c//128 of the [2048, D] scratch view.
            cki = pp.tile([QW, FW], I32)
            nc.vector.tensor_copy(cki[:], ck[:])
            chi = pp.tile([QW, FW], I32)
            nc.vector.tensor_scalar(chi[:], cki[:], 7, None,
                                    mybir.AluOpType.logical_shift_right)
            nc.vector.tensor_scalar(cki[:], cki[:], 127, None,
                                    mybir.AluOpType.bitwise_and)
            nc.vector.tensor_scalar(cki[:], cki[:], 4, None,
                                    mybir.AluOpType.logical_shift_left)
            nc.vector.tensor_tensor(cki[:], cki[:], chi[:],
                                    mybir.AluOpType.add)
            ck16 = pp.tile([QW, FW], I16)
            ck16_cp = nc.vector.tensor_copy(ck16[:], cki[:])
            # replicate to all 8 GPSIMD core groups (cross-partition copies)
            ckrep = pp.tile([P, FW], I16)
            for cgrp in range(P // QW):
                nc.scalar.dma_start(ckrep[cgrp * QW:(cgrp + 1) * QW, :], ck16[:])

            # prepared output gathers: desc-gen on GpSimd right after the p
            # gather (only needs ckrep); the DMAs fire at the triggers below.
            # sm_scratch is a raw (untracked) DRAM tensor so the preps do not
            # inherit the smoothed-writeback RAW dep -- that dep is carried
            # manually by the triggers. Layout is p-major: [p, ci, d].
            sm_dram = nc.dram_tensor("sm_scratch", [P, NCB, D], BF16,
                                     kind="Internal")
            sm_flat = sm_dram[:].rearrange("p ci d -> (p ci) d")
            gs = []

            def emit_prep(k):
                g = gp.tile([P, IPC // P, D], BF16, tag="g")
                nc.gpsimd.dma_gather(
                    out_ap=g[:], in_ap=sm_flat,
                    idxs_ap=ckrep[:, k * (FW // NGC):(k + 1) * (FW // NGC)],
                    num_idxs=IPC, num_idxs_reg=IPC, elem_size=D,
                    prepare_only=True, sem=dma_sems[k])
                gs.append(g)

            # GpSimd stream order == emission order: the first 2 preps run
            # while the scan pipeline computes; the remaining preps are
            # emitted between triggers so desc-gen overlaps phase-3 DMA
            # instead of stalling behind writeback sem waits.
            emit_prep(0)
            emit_prep(1)
            psone_cm.__exit__(None, None, None)

            # ---------------- p row -> PSUM broadcast (per half) ----------
            # Each half's chain (clip -> transpose -> DRAM bounce -> ones
            # matmul broadcast -> decay) starts as soon as ITS p-gather call
            # lands, so the half-A scans begin ~8us after pg0's data.
            pbp_cm = tc.tile_pool(name="pbh", bufs=1, space="PSUM")
            pbp = pbp_cm.__enter__()
            pb_ps = pbp.tile([P, C], F32, space="PSUM", tag="pb")
            db = pp.tile([P, C], BF16)
            prow = pp.tile([1, C], F32)
            ones128 = pp.tile([1, P], F32)
            nc.vector.memset(ones128[:], 1.0)
            p_dram = nc.dram_tensor("p_row_scratch", [1, C], F32, kind="Internal")
            NJH = NCB // 2

            def p_half(h):
                lo, hi = h * CH, (h + 1) * CH
                p2h = p2big[:, h * NJH:(h + 1) * NJH, 0:1].rearrange(
                    "p j o -> p (j o)")   # [128, 8]
                cl = nc.vector.tensor_scalar(p2h, p2h, EPS, 1.0 - EPS,
                                             mybir.AluOpType.max,
                                             mybir.AluOpType.min)
                cl._wait_ge(psems[h], 16)
                # no-sync edge: the clip's sem wait stalls the DVE stream, so
                # keep Tile from hoisting it ahead of the chunk-id pipeline
                # (which gates the gather preps).
                bass._add_dep_helper(cl.ins, ck16_cp.ins, sync=False,
                                     reason="clip after ck pipeline")
                if h == 0:
                    nc.vector.memset(p2big[0:1, 0:1, 0:1], 1.0)  # d[0]=0
                ptps = psp.tile([NJH, P], F32, space="PSUM", tag="ptps")
                nc.tensor.transpose(ptps[:], p2h, ident[:])
                pTh = pp.tile([NJH, P], F32, tag=f"pt{h}")
                nc.scalar.copy(pTh[:], ptps[:])
                w = nc.scalar.dma_start(
                    p_dram[:, lo:hi].rearrange("o (j q) -> (o j) q", j=NJH),
                    pTh[:])
                r = nc.scalar.dma_start(prow[:, lo:hi], p_dram[:, lo:hi])
                bass._add_dep_helper(r.ins, w.ins, sync=True,
                                     reason="p row bounce raw")
                for n in range(lo // 512, hi // 512):
                    nc.tensor.matmul(pb_ps[:, n * 512:(n + 1) * 512],
                                     lhsT=ones128[:],
                                     rhs=prow[:, n * 512:(n + 1) * 512],
                                     start=True, stop=True)
                nc.scalar.activation(db[:, lo:hi], pb_ps[:, lo:hi], COPY,
                                     bias=1.0, scale=-1.0)

            # ---------------- EMA scan, c-half-major with carry fold -------
            # half A (chunks < 1024) completes for all 4 d-groups first so
            # its writeback (and gather call 0) can fire early.
            sm_sb = pp.tile([P, NCB * D], BF16)  # [c-in-block, (c-block, d)]
            wts, sts = [], []
            for dg in range(NDG):
                wt = pp.tile([P, C], BF16, tag=f"wt{dg}")
                st = pp.tile([P, C], BF16, tag=f"st{dg}")
                wts.append(wt)
                sts.append(st)

            def transpose_out(dg, ci, engine):
                ps = psp.tile([P, P], BF16, space="PSUM", tag="tps")
                nc.tensor.transpose(ps[:], sts[dg][:, ci * P:(ci + 1) * P],
                                    ident_bf[:])
                dst = sm_sb[:, ci * D + dg * P: ci * D + (dg + 1) * P]
                engine(dst, ps[:])

            def scan_half(half):
                lo, hi = half * CH, (half + 1) * CH
                for dg in range(NDG):
                    wt, st = wts[dg], sts[dg]
                    nc.vector.tensor_tensor(wt[:, lo:hi], hts[dg][:, lo:hi],
                                            pb_ps[:, lo:hi],
                                            mybir.AluOpType.mult)
                    if half == 1:
                        # fold the half-A carry into the first weighted col
                        carry = pp.tile([P, 1], F32, tag=f"cr{dg}")
                        nc.vector.tensor_tensor(
                            carry[:], st[:, CH - 1:CH], db[:, CH:CH + 1],
                            mybir.AluOpType.mult)
                        nc.vector.tensor_tensor(
                            wt[:, CH:CH + 1], wt[:, CH:CH + 1], carry[:],
                            mybir.AluOpType.add)
                    nc.vector.tensor_tensor_scan(
                        st[:, lo:hi], db[:, lo:hi], wt[:, lo:hi], 0.0,
                        mybir.AluOpType.mult, mybir.AluOpType.add)
                    # transpose this (half, dg)'s 8 c-blocks while the next
                    # scan runs; copies go to Scalar (DVE keeps scanning),
                    # except half 1's last d-group splits onto DVE (free
                    # after the final scan) so wbB isn't scalar-gated.
                    for ci in range(half * (NCB // 2), (half + 1) * (NCB // 2)):
                        if half == 1 and dg == NDG - 1 and ci % 2 == 0:
                            transpose_out(dg, ci, nc.vector.tensor_copy)
                        else:
                            transpose_out(dg, ci, nc.scalar.copy)
                    if half == 0 and dg == 0:
                        # half B's p chain (clip/bounce/broadcast) overlaps
                        # the remaining half-A scans
                        p_half(1)
                # writeback this half: 2D flat APs -> one contiguous 8KB run
                # per partition on both sides (big descriptors)
                hb, he = half * (NCB // 2), (half + 1) * (NCB // 2)
                return nc.sync.dma_start(
                    sm_dram[:].rearrange("p ci d -> p (ci d)")[:, hb * D:he * D],
                    sm_sb[:, hb * D:he * D])

            p_half(0)
            w_sm_a = scan_half(0)
            w_sm_b = scan_half(1)

            # ---------------- output expansion ----------------
            # triggers fire each prepared gather once the smoothed data it
            # can reference has landed: call 0 only touches chunks < 1024
            # (chunk_id[l] <= l), later calls may touch anything.
            trig_of = []

            def emit_trigger(count, need_b):
                tr = nc.gpsimd.trigger_dma(count=count)
                bass._add_dep_helper(tr.ins, w_sm_a.ins, sync=True,
                                     reason="smoothed gather raw a")
                if need_b:
                    bass._add_dep_helper(tr.ins, w_sm_b.ins, sync=True,
                                         reason="smoothed gather raw b")
                trig_of.extend([tr] * count)
                return tr

            emit_trigger(1, need_b=False)   # call 0
            emit_prep(2)
            emit_prep(3)
            emit_trigger(3, need_b=True)    # calls 1-3
            emit_prep(4)
            emit_trigger(1, need_b=True)    # call 4
            emit_prep(5)
            emit_trigger(1, need_b=True)    # call 5
            emit_prep(6)
            emit_trigger(1, need_b=True)    # call 6
            emit_prep(7)
            emit_trigger(1, need_b=True)    # call 7

            for k in range(NGC):
                # Tile's auto DMASW wait is satisfied by the prep-time
                # pre-bump, not the gather's completion; the baked per-call
                # sem is the real data-ready signal. The no-sync edge on the
                # trigger keeps the scheduler from hoisting this wait ahead
                # of the phase-1 work on its engine.
                # 2D [128, 4096] APs on both sides: one contiguous 8KB run
                # per partition, so the AP normalizer emits few big
                # descriptors instead of 1024 per-(p,g) 1KB ones. Alternate
                # the two HWDGE rings (sync=SP, scalar=ACT) -- HWDGE DMAs
                # are FIFO per ring, so one ring would serialize all 8.
                GW = (IPC // P) * D
                eng = nc.sync if k % 2 == 0 else nc.scalar
                od = eng.dma_start(
                    out_d[:].rearrange("p g d -> p (g d)")[:, k * GW:(k + 1) * GW],
                    gs[k][:].rearrange("p g d -> p (g d)"))
                od._wait_ge(dma_sems[k], 16)
                bass._add_dep_helper(od.ins, trig_of[k].ins, sync=False,
                                     reason="out write after trigger")
            pbp_cm.__exit__(None, None, None)

    nc.compile()
    return nc


def _shard_inputs(hidden_states, boundary_mask, boundary_prob, take_idx):
    import ml_dtypes
    hidden_states = np.asarray(hidden_states, dtype=np.float32)
    boundary_mask = np.asarray(boundary_mask)
    boundary_prob = np.asarray(boundary_prob, dtype=np.float32)
    take_idx = np.asarray(take_idx)
    ident = np.eye(P, dtype=np.float32)
    identbf = np.eye(P).astype(ml_dtypes.bfloat16)
    uti = np.triu(np.ones((QW, QW), dtype=np.float32))
    in_maps = []
    for b in range(B):
        probp = np.zeros((L, PE), dtype=np.float32)
        probp[:, 0] = boundary_prob[b]
        # tidx16[chan, col] = take_idx[16*col + chan], replicated to all 8
        # GPSIMD core groups
        t16 = take_idx[b].astype(np.int16).reshape(C // QW, QW).T
        in_maps.append({
            "ident": ident, "identbf": identbf, "uti": uti,
            "hidden_t": np.ascontiguousarray(hidden_states[b].T),
            # maskw[q, j] = mask[16j + q]
            "maskw": np.ascontiguousarray(
                boundary_mask[b].astype(np.uint8).reshape(FW, QW).T),
            "probp": probp,
            "tidx16": np.ascontiguousarray(np.tile(t16, (P // QW, 1))),
        })
    return in_maps


last_results = None  # populated by kernel() for profiling harnesses


def kernel(hidden_states, boundary_mask, boundary_prob, take_idx,
           **run_kwargs) -> np.ndarray:
    global _CACHED_NC, last_results
    if _CACHED_NC is None:
        _CACHED_NC = build_nc()
    in_maps = _shard_inputs(hidden_states, boundary_mask, boundary_prob, take_idx)
    res = run_bass_kernel_spmd(_CACHED_NC, in_maps, core_ids=list(range(B)),
                               **run_kwargs)
    last_results = res
    outs = []
    for b in range(B):
        x = np.asarray(res.results[b]["out"])      # [128, 64, 512] bf16
        outs.append(x.transpose(1, 0, 2).reshape(L, D))
    return np.stack(outs, axis=0).astype(np.float32)
